# revision 2
# baseline (speedup 1.0000x reference)
"""Trainium2 Bass kernel for the wf-psf TF_physical_poly_field forward model.

8 NeuronCores, data-parallel over the 32-star batch (4 stars/core).

Host prep (tiny, O(B*K) math):
  - exact-position match + polynomial features -> per-star coefficient row
    C[s, 0:87] over 87 basis maps (66 zernikes + 21 alpha-folded S rows).
  - basis maps pre-masked by the pupil obscuration and transposed:
    W'[k, y*256+x] = (map_k * obsc)[x, y]   (fp16)
  - per-bin DFT tables: the reference zero-pads the 256x256 pupil to pN_j,
    FFTs, fftshifts and keeps the centred 96x96 crop.  |FFT|^2 is invariant
    to the zero-pad shift, so the crop equals A = E_j P E_j^T with
    E_j[f, y] = exp(-2pi i f y / pN_j), f in [-48, 48).  Tables C/S/negS =
    cos/sin/-sin(2pi f y / pN_j), shape (256, 96), fp16.
  - obscuration correction D_j = -E_j (1-obsc) E_j^T (from the fp16-rounded
    tables, split hi+lo fp16) so the device never multiplies by the mask:
    it computes P' = exp(i k opd_masked) (= P inside pupil, = 1 outside)
    and matmul-injects D to cancel the outside region exactly.

Device per core:
  1. opd: stream W' chunks, matmul with C^T (K=87), bounce through DRAM
     scratch to reshape each star's flat opd into (y-partition, x-free) fp16.
  2. per (star, bin): fp16-magic range reduction (Sin table covers [-pi,pi]
     only), two Sin activations -> exp(i k opd) fp16, two DFT matmul stages
     -> A (96x96 complex), Square + 3x3 sum-pool (strided adds + pooling
     matmul whose leading ones-column also emits flux totals), fused
     scalar_tensor_tensor accumulates sed/total * pooled into the PSF.
"""

import numpy as np

import concourse.bacc as bacc
import concourse.tile as tile
from concourse import mybir
from concourse.bass_utils import run_bass_kernel_spmd

F32 = mybir.dt.float32
F16 = mybir.dt.float16
AF = mybir.ActivationFunctionType
ALU = mybir.AluOpType

# ---- static model configuration (mirrors the reference driver args) ----
BATCH = 32
N_ZKS_TOTAL = 66
N_ZKS_PARAM = 45
D_MAX = 2
D_MAX_NP = 5
OPD_DIM = 256
N_BINS = 20
OUTPUT_DIM = 32
OVERSAMPLING = 3.0
LAMBDAS = np.linspace(0.55, 0.9, N_BINS)
PHASE_NS = [int(2 * round(OPD_DIM * OVERSAMPLING * l / (2.0 * LAMBDAS[0])))
            for l in LAMBDAS]
N_CORES = 8
SPC = BATCH // N_CORES          # stars per core
KMAT = N_ZKS_TOTAL + 21         # 87 basis maps
CROP = 96                       # 96x96 centre crop of the FFT
NPIX = OPD_DIM * OPD_DIM

MAGIC = 1536.0                  # fp16 round-to-int magic (quantum 1.0 there)
HALF_PI = float(np.pi / 2)

LAM32 = [float(np.float32(l)) for l in LAMBDAS]
KVAL = [float(np.float32(2.0 * np.pi) / np.float32(l)) for l in LAMBDAS]

OPD_CHUNK = 2048
N_CHUNKS = NPIX // OPD_CHUNK    # 32


def _poly_pos_mat(positions, d_max):
    """fp32 Mendel-ordered polynomial position matrix, shape (n_poly, B)."""
    x = positions[:, 0] / np.float32(1000.0) * np.float32(2.0) - np.float32(1.0)
    y = positions[:, 1] / np.float32(1000.0) * np.float32(2.0) - np.float32(1.0)
    cols = []
    for d in range(d_max + 1):
        for p in range(d + 1):
            cols.append((x ** (d - p)) * (y ** p))
    return np.stack(cols, axis=0).astype(np.float32)


def _host_prep(positions, packed_SED_data, coeff_mat, alpha_mat, S_mat,
               zernike_maps, obscurations, obs_pos, zks_prior):
    pos = np.asarray(positions, np.float32)

    pm = _poly_pos_mat(pos, D_MAX)                          # (6, B)
    zk_param = (np.asarray(coeff_mat, np.float32) @ pm).T   # (B, 45)
    eq = (pos[:, None, :] == np.asarray(obs_pos, np.float32)[None, :, :]).all(-1)
    idx = eq.argmax(1)
    zks = np.asarray(zks_prior, np.float32)[idx].copy()     # (B, 66)
    zks[:, :N_ZKS_PARAM] += zk_param

    pm_np = _poly_pos_mat(pos, D_MAX_NP)                    # (21, B)
    beta = pm_np.T @ np.asarray(alpha_mat, np.float32)      # (B, 21)
    C = np.concatenate([zks, beta], axis=1)                 # (B, 87)

    obsc = np.asarray(obscurations, np.float32)
    W = np.concatenate([np.asarray(zernike_maps, np.float32),
                        np.asarray(S_mat, np.float32)], axis=0)
    Wm = W * obsc[None, :, :]
    Wt = np.ascontiguousarray(
        Wm.transpose(0, 2, 1).reshape(KMAT, NPIX)).astype(np.float16)

    f = np.arange(CROP, dtype=np.float64) - CROP // 2
    y = np.arange(OPD_DIM, dtype=np.float64)
    # stage-1 rhs tables: per y-tile, per bin 192 cols: taba = [C | -S] (for Pr),
    # tabb = [S | C] (for Pi) -> one N=192 matmul accumulates [Ur | Ui].
    taba = np.empty((2, 128, N_BINS * 192), np.float16)
    tabb = np.empty_like(taba)
    # stage-2 lhsT tables, padded to 128 cols (FWL): [C|0], [S|0], [-S|0]
    cpad = np.zeros((2, 128, N_BINS * 128), np.float16)
    spad = np.zeros_like(cpad)
    nspad = np.zeros_like(cpad)
    dthi = np.zeros((CROP, N_BINS * 2 * 128), np.float16)
    dtlo = np.zeros_like(dthi)
    m1 = (1.0 - obsc).astype(np.float64)
    for j in range(N_BINS):
        ang = 2.0 * np.pi * np.outer(y, f) / PHASE_NS[j]    # (256, 96)
        c16 = np.cos(ang).astype(np.float16)
        s16 = np.sin(ang).astype(np.float16)
        for t in range(2):
            rows = slice(t * 128, (t + 1) * 128)
            taba[t, :, j * 192:j * 192 + 96] = c16[rows]
            taba[t, :, j * 192 + 96:(j + 1) * 192] = -s16[rows]
            tabb[t, :, j * 192:j * 192 + 96] = s16[rows]
            tabb[t, :, j * 192 + 96:(j + 1) * 192] = c16[rows]
            cpad[t, :, j * 128:j * 128 + 96] = c16[rows]
            spad[t, :, j * 128:j * 128 + 96] = s16[rows]
            nspad[t, :, j * 128:j * 128 + 96] = -s16[rows]
        Eh = (c16.astype(np.float64) - 1j * s16.astype(np.float64)).T  # (96,256)
        D = -(Eh @ m1 @ Eh.T)                               # (96, 96) complex
        for part, Dp in ((0, D.real), (1, D.imag)):
            DT = Dp.T                                       # lhsT layout
            hi = DT.astype(np.float16)
            lo = (DT - hi.astype(np.float64)).astype(np.float16)
            col = (2 * j + part) * 128
            dthi[:, col:col + CROP] = hi
            dtlo[:, col:col + CROP] = lo

    i4 = np.zeros((CROP, SPC * CROP), np.float16)
    for s in range(SPC):
        i4[:, s * CROP:(s + 1) * CROP] = np.eye(CROP, dtype=np.float16)

    # partition-pooling matmul (3->1) and all-ones totals matmul
    qt32 = np.zeros((CROP, 32), np.float32)
    for k in range(CROP):
        qt32[k, k // 3] = 1.0
    ones96 = np.ones((CROP, 32), np.float32)

    sed = np.asarray(packed_SED_data, np.float32)[:, :, 2]  # (B, 20)
    return (C, Wt, taba, tabb, cpad, spad, nspad, dthi, dtlo, i4, qt32,
            ones96, sed)


def _build_nc(repeat=1):
    nc = bacc.Bacc("TRN2", target_bir_lowering=False)

    cmat = nc.dram_tensor("cmat", [KMAT, SPC], F16, kind="ExternalInput")
    wmat = nc.dram_tensor("wmat", [KMAT, NPIX], F16, kind="ExternalInput")
    taba_d = nc.dram_tensor("taba", [2, 128, N_BINS * 192], F16,
                            kind="ExternalInput")
    tabb_d = nc.dram_tensor("tabb", [2, 128, N_BINS * 192], F16,
                            kind="ExternalInput")
    cpad_d = nc.dram_tensor("cpad", [2, 128, N_BINS * 128], F16,
                            kind="ExternalInput")
    spad_d = nc.dram_tensor("spad", [2, 128, N_BINS * 128], F16,
                            kind="ExternalInput")
    nspad_d = nc.dram_tensor("nspad", [2, 128, N_BINS * 128], F16,
                             kind="ExternalInput")
    dthi_d = nc.dram_tensor("dthi", [CROP, N_BINS * 2 * 128], F16,
                            kind="ExternalInput")
    dtlo_d = nc.dram_tensor("dtlo", [CROP, N_BINS * 2 * 128], F16,
                            kind="ExternalInput")
    i4_d = nc.dram_tensor("i4", [CROP, SPC * CROP], F16, kind="ExternalInput")
    qt32_d = nc.dram_tensor("qt32", [CROP, 32], F32, kind="ExternalInput")
    ones_d = nc.dram_tensor("ones96", [CROP, 32], F32, kind="ExternalInput")
    sed_d = nc.dram_tensor("sed", [32, SPC * N_BINS], F32, kind="ExternalInput")
    psf_out = nc.dram_tensor("psf_out", [SPC, OUTPUT_DIM, OUTPUT_DIM], F32,
                             kind="ExternalOutput")
    opd_scr = nc.dram_tensor("opd_scr", [SPC, NPIX], F16)   # internal scratch

    with tile.TileContext(nc) as tc:
        with tc.tile_pool(name="const", bufs=1) as cpool:
            halfpi = cpool.tile([128, 1], F32)
            nc.gpsimd.memset(halfpi[:], HALF_PI)
            c_sb = cpool.tile([KMAT, SPC], F16)
            nc.sync.dma_start(c_sb[:], cmat[:])
            taba_sb = [cpool.tile([128, N_BINS * 192], F16, name=f"taba{t}",
                                  tag=f"ta{t}") for t in range(2)]
            tabb_sb = [cpool.tile([128, N_BINS * 192], F16, name=f"tabb{t}",
                                  tag=f"tb{t}") for t in range(2)]
            cpad_sb = [cpool.tile([128, N_BINS * 128], F16, name=f"cpad{t}",
                                  tag=f"cp{t}") for t in range(2)]
            spad_sb = [cpool.tile([128, N_BINS * 128], F16, name=f"spad{t}",
                                  tag=f"sp{t}") for t in range(2)]
            nspad_sb = [cpool.tile([128, N_BINS * 128], F16, name=f"nspad{t}",
                                   tag=f"np{t}") for t in range(2)]
            for t in range(2):
                nc.sync.dma_start(taba_sb[t][:], taba_d[t])
                nc.sync.dma_start(tabb_sb[t][:], tabb_d[t])
                nc.sync.dma_start(cpad_sb[t][:], cpad_d[t])
                nc.sync.dma_start(spad_sb[t][:], spad_d[t])
                nc.sync.dma_start(nspad_sb[t][:], nspad_d[t])
            dthi_sb = cpool.tile([CROP, N_BINS * 2 * 128], F16)
            nc.sync.dma_start(dthi_sb[:], dthi_d[:])
            dtlo_sb = cpool.tile([CROP, N_BINS * 2 * 128], F16)
            nc.sync.dma_start(dtlo_sb[:], dtlo_d[:])
            i4_sb = cpool.tile([CROP, SPC * CROP], F16)
            nc.sync.dma_start(i4_sb[:], i4_d[:])
            qt32_sb = cpool.tile([CROP, 32], F32)
            nc.sync.dma_start(qt32_sb[:], qt32_d[:])
            ones_sb = cpool.tile([CROP, 32], F32)
            nc.sync.dma_start(ones_sb[:], ones_d[:])
            sed_sb = cpool.tile([32, SPC * N_BINS], F32)
            nc.sync.dma_start(sed_sb[:], sed_d[:])
            opd16 = cpool.tile([128, SPC * 512], F16)   # (y, x) per star
            psf_all = cpool.tile([32, SPC * 32], F32)
            nc.gpsimd.memset(psf_all[:], 0.0)

            import contextlib
            rep_ctx = (tc.For_i(0, repeat, 1, hint_engines=tuple(nc.engines))
                       if repeat > 1 else contextlib.nullcontext())
            with rep_ctx:
                # ---- opd phase ----
                with tc.tile_pool(name="wpool", bufs=2) as wpool, \
                     tc.tile_pool(name="opd_ps", bufs=2, space="PSUM") as opd_ps, \
                     tc.tile_pool(name="chpool", bufs=3) as chpool:
                    for ci in range(N_CHUNKS):
                        wc = wpool.tile([KMAT, OPD_CHUNK], F16, tag="wc")
                        nc.sync.dma_start(
                            wc[:], wmat[:, ci * OPD_CHUNK:(ci + 1) * OPD_CHUNK])
                        ops = opd_ps.tile([SPC, OPD_CHUNK], F32, tag="ops")
                        for mi in range(OPD_CHUNK // 512):
                            nc.tensor.matmul(ops[:, mi * 512:(mi + 1) * 512],
                                             c_sb[:], wc[:, mi * 512:(mi + 1) * 512],
                                             start=True, stop=True)
                        ch = chpool.tile([SPC, OPD_CHUNK], F16, tag="ch")
                        if ci % 2 == 0:
                            nc.scalar.copy(ch[:], ops[:])
                        else:
                            nc.vector.tensor_copy(ch[:], ops[:])
                        nc.sync.dma_start(
                            opd_scr[:, ci * OPD_CHUNK:(ci + 1) * OPD_CHUNK], ch[:])
                    for s in range(SPC):
                        for t in range(2):
                            src = opd_scr[s, t * 32768:(t + 1) * 32768]
                            nc.sync.dma_start(
                                opd16[:, s * 512 + t * 256: s * 512 + (t + 1) * 256],
                                src.rearrange("(p f) -> p f", p=128))

                # ---- main loop ----
                with tc.tile_pool(name="elw", bufs=5) as elw, \
                     tc.tile_pool(name="usb", bufs=3) as usbp, \
                     tc.tile_pool(name="sqp", bufs=3) as sqp, \
                     tc.tile_pool(name="tailp", bufs=3) as tailp, \
                     tc.tile_pool(name="u_ps", bufs=3, space="PSUM") as u_ps, \
                     tc.tile_pool(name="a_ps", bufs=1, space="PSUM") as a_ps, \
                     tc.tile_pool(name="pool_ps", bufs=2, space="PSUM") as pool_ps, \
                     tc.tile_pool(name="tot_ps", bufs=1, space="PSUM") as tot_ps:
                    for j in range(N_BINS):
                        lam = LAM32[j]
                        kj = KVAL[j]
                        cs = slice(j * CROP, (j + 1) * CROP)
                        # A tile: per-star stride 256 cols keeps each (96,96) matmul
                        # output inside one PSUM bank (512 fp32 per bank).
                        a_all = a_ps.tile([128, SPC * 256], F32, tag="a")
                        usb = [usbp.tile([128, SPC * 192], F16, name=f"usb{t}_{j}",
                                         tag=f"u{t}") for t in range(2)]
                        for s in range(SPC):
                            opd_s = opd16[:, s * 512:(s + 1) * 512]
                            r16 = elw.tile([128, 512], F16, tag="r16")
                            nc.vector.tensor_scalar(r16[:], opd_s, 1.0 / lam,
                                                    MAGIC, op0=ALU.mult,
                                                    op1=ALU.add)
                            rr = elw.tile([128, 512], F16, tag="rr")
                            nc.vector.tensor_scalar(rr[:], r16[:], -MAGIC, None,
                                                    op0=ALU.add)
                            th = elw.tile([128, 512], F16, tag="th")
                            nc.vector.scalar_tensor_tensor(th[:], rr[:], -lam,
                                                           opd_s, op0=ALU.mult,
                                                           op1=ALU.add)
                            av = elw.tile([128, 512], F16, tag="av")
                            nc.vector.scalar_tensor_tensor(av[:], th[:], -1.0,
                                                           th[:], op0=ALU.mult,
                                                           op1=ALU.max)
                            pim = elw.tile([128, 512], F16, tag="pim")
                            nc.scalar.activation(pim[:], th[:], AF.Sin,
                                                 bias=0.0, scale=kj)
                            pre = elw.tile([128, 512], F16, tag="pre")
                            nc.scalar.activation(pre[:], av[:], AF.Sin,
                                                 bias=halfpi[:], scale=-kj)

                            # stage 1: paired-table rhs [C|-S]/[S|C] -> one
                            # N=192 matmul accumulates [Ur | Ui] per weight
                            ups = [u_ps.tile([128, 192], F32,
                                             name=f"ups{_t}_{j}_{s}",
                                             tag="ups") for _t in range(2)]
                            s1 = slice(j * 192, (j + 1) * 192)
                            for xt in range(2):
                                for yi, yt in enumerate((0, 1)):
                                    prs = pre[:, 256 * yt + 128 * xt:
                                              256 * yt + 128 * (xt + 1)]
                                    pis = pim[:, 256 * yt + 128 * xt:
                                              256 * yt + 128 * (xt + 1)]
                                    nc.tensor.matmul(ups[xt][:], prs,
                                                     taba_sb[yt][:, s1],
                                                     start=(yi == 0),
                                                     stop=False)
                                    nc.tensor.matmul(ups[xt][:], pis,
                                                     tabb_sb[yt][:, s1],
                                                     start=False,
                                                     stop=(yi == 1))
                            nc.scalar.copy(usb[0][:, 192 * s:192 * (s + 1)],
                                           ups[0][:])
                            nc.vector.tensor_copy(
                                usb[1][:, 192 * s:192 * (s + 1)], ups[1][:])

                            # stage 2: A = E @ U + D, one psum group per star
                            a_s = a_all[:, 256 * s:256 * s + 192]
                            are = a_all[:, 256 * s:256 * s + 96]
                            aim = a_all[:, 256 * s + 96:256 * s + 192]
                            uboth = [usb[xt][:, 192 * s:192 * (s + 1)]
                                     for xt in range(2)]
                            ur = [usb[xt][:, 192 * s:192 * s + 96]
                                  for xt in range(2)]
                            ui = [usb[xt][:, 192 * s + 96:192 * (s + 1)]
                                  for xt in range(2)]
                            islc = i4_sb[:, s * CROP:(s + 1) * CROP]
                            s2 = slice(j * 128, (j + 1) * 128)
                            dre = slice((2 * j) * 128, (2 * j + 1) * 128)
                            dim = slice((2 * j + 1) * 128, (2 * j + 2) * 128)
                            nc.tensor.matmul(a_s, cpad_sb[0][:, s2], uboth[0],
                                             start=True, stop=False)
                            nc.tensor.matmul(a_s, cpad_sb[1][:, s2], uboth[1],
                                             start=False, stop=False)
                            nc.tensor.matmul(are, spad_sb[0][:, s2], ui[0],
                                             start=False, stop=False)
                            nc.tensor.matmul(are, spad_sb[1][:, s2], ui[1],
                                             start=False, stop=False)
                            nc.tensor.matmul(aim, nspad_sb[0][:, s2], ur[0],
                                             start=False, stop=False)
                            nc.tensor.matmul(aim, nspad_sb[1][:, s2], ur[1],
                                             start=False, stop=False)
                            nc.tensor.matmul(are, dthi_sb[:, dre], islc,
                                             start=False, stop=False)
                            nc.tensor.matmul(are, dtlo_sb[:, dre], islc,
                                             start=False, stop=False)
                            nc.tensor.matmul(aim, dthi_sb[:, dim], islc,
                                             start=False, stop=False)
                            nc.tensor.matmul(aim, dtlo_sb[:, dim], islc,
                                             start=False, stop=True)

                        # ---- bin tail (batched over the 4 stars) ----
                        sq = sqp.tile([CROP, SPC * 192], F32, tag="sq")
                        av4 = a_all[0:CROP, :].rearrange("p (s g) -> p s g", g=256)
                        nc.scalar.activation(
                            sq[:].rearrange("p (s g) -> p s g", g=192),
                            av4[:, :, 0:192], AF.Square)
                        ps_all = sqp.tile([CROP, SPC * 96], F32, tag="ps")
                        sq4 = sq[:].rearrange("p (s h g) -> p s h g", h=2, g=96)
                        nc.vector.tensor_tensor(
                            ps_all[:].rearrange("p (s g) -> p s g", g=96),
                            sq4[:, :, 0, :], sq4[:, :, 1, :], op=ALU.add)
                        ps1 = tailp.tile([CROP, 132], F32, tag="ps1")
                        pv = ps_all[:].rearrange("p (s q c) -> p s q c", q=32, c=3)
                        t1 = tailp.tile([CROP, 128], F32, tag="t1")
                        nc.vector.tensor_tensor(
                            t1[:].rearrange("p (s q) -> p s q", q=32),
                            pv[:, :, :, 0], pv[:, :, :, 1], op=ALU.add)
                        nc.vector.tensor_tensor(
                            ps1[:, 0:128].rearrange("p (s q) -> p s q", q=32),
                            t1[:].rearrange("p (s q) -> p s q", q=32),
                            pv[:, :, :, 2], op=ALU.add)
                        nc.vector.tensor_reduce(
                            ps1[:, 128:132],
                            ps1[:, 0:128].rearrange("p (s q) -> p s q", s=SPC),
                            axis=mybir.AxisListType.X, op=ALU.add)
                        plp = pool_ps.tile([32, 128], F32, tag="plp")
                        nc.tensor.matmul(plp[:], qt32_sb[:], ps1[:, 0:128],
                                         start=True, stop=True)
                        totp = tot_ps.tile([32, SPC], F32, tag="totp")
                        nc.tensor.matmul(totp[:], ones_sb[:], ps1[:, 128:132],
                                         start=True, stop=True)
                        plsb = tailp.tile([32, 128], F32, tag="plsb")
                        nc.scalar.copy(plsb[:], plp[:])
                        rcp = tailp.tile([32, SPC], F32, tag="rcp")
                        nc.vector.reciprocal(rcp[:], totp[:])
                        scl = tailp.tile([32, SPC], F32, tag="scl")
                        nc.vector.tensor_tensor(
                            scl[:], rcp[:], sed_sb[:, j * SPC:(j + 1) * SPC],
                            op=ALU.mult)
                        for s in range(SPC):
                            dst = psf_all[:, 32 * s:32 * (s + 1)]
                            nc.vector.scalar_tensor_tensor(
                                dst, plsb[:, 32 * s:32 * (s + 1)],
                                scl[:, s:s + 1], dst,
                                op0=ALU.mult, op1=ALU.add)

                    for s in range(SPC):
                        nc.gpsimd.dma_start(psf_out[s],
                                            psf_all[:, 32 * s:32 * (s + 1)])

    nc.compile()
    return nc


_NC_CACHE = []


def _make_in_maps(inputs):
    (C, Wt, taba, tabb, cpad, spad, nspad, dthi, dtlo, i4, qt32, ones96,
     sed) = _host_prep(**inputs)
    shared = {
        "wmat": Wt, "taba": taba, "tabb": tabb, "cpad": cpad, "spad": spad,
        "nspad": nspad, "dthi": dthi, "dtlo": dtlo, "i4": i4, "qt32": qt32,
        "ones96": ones96,
    }
    in_maps = []
    for c in range(N_CORES):
        sl = slice(c * SPC, (c + 1) * SPC)
        sed_row = np.broadcast_to(
            sed[sl].T.reshape(1, N_BINS * SPC), (32, N_BINS * SPC))
        sed_row = np.ascontiguousarray(sed_row).astype(np.float32)
        in_maps.append(dict(
            shared,
            cmat=np.ascontiguousarray(C[sl].T).astype(np.float16),
            sed=sed_row,
        ))
    return in_maps


def kernel(**inputs):
    if not _NC_CACHE:
        _NC_CACHE.append(_build_nc())
    nc = _NC_CACHE[0]
    in_maps = _make_in_maps(inputs)
    res = run_bass_kernel_spmd(nc, in_maps, core_ids=list(range(N_CORES)))
    out = np.concatenate([r["psf_out"] for r in res.results], axis=0)
    return out.astype(np.float32)

